# revision 30
# baseline (speedup 1.0000x reference)
"""BatchHardTripletLoss kernel for 8 Trainium2 NeuronCores.

Math (matches the jax reference):
  dist2[i,j] = |e1_i|^2 + |e2_j|^2 - 2 e1.e2 + 2*eps*(s1_i - s2_j) + D*eps^2
             = a[i] + v[i,j],   v[i,j] = b[j] - 2<e1_i, e2_j>
  pos_max[i] = sqrt(clip(a[i] + max_{j in pos} v[i,j], 0))
  neg_min[i] = sqrt(clip(a[i] + min_{j in neg} v[i,j], 0))
  loss = mean over POS anchors of relu(pos_max - neg_min + margin)

v3 architecture: PAIRWISE TOURNAMENT + DoubleRow bias folding.
The drain of the [anchors x cands] f32 matrix out of PSUM (DVE/Act at
~1 elem/cyc/partition) is the wall, so candidates are paired on the
host:  max(d_u, d_v) = d_v + relu(d_u - d_v), and d_u - d_v =
(b_u - b_v) - 2<e1, e2_u - e2_v> is ONE matmul column.  Per PSUM
group of 1024 pair-columns:
  phase A: fp8 DoubleRow mains (virtual K=256: 128 embedding dims on
           the i=0 plane, 4-term fp8 bias split on rows 0-3 of the
           i=1 plane against ones in the lhsT) -> diff + bias-diff
  Act:     relu in-place on the PSUM group (only TensorE touches
           has_written, so the accumulate below still works)
  phase B: fp8 DoubleRow mains, start=False -> accumulate base
           d_v (+ its bias) on top of relu(d_u - d_v)
  DVE:     one chained tensor_scalar max-accum per class segment
This halves the reduced stream (4096 pair-cols vs 8192 cols per
i-tile).  Neg class is sign-flipped so both classes are MAX chains.
A-phases are emitted one group ahead of B-phases so the PE FIFO never
head-of-line blocks on a relu.  4 PSUM groups (2 banks each) rotate.

Host: pos-first sort, exact f64 stats, pairing (self-pair for odd
class tails, fp8-saturated -BIG dummy pad to 4096 pairs, odd columns
peeled into an exact host-side merge), packing, final sqrt/margin/mean
+ exact f64 remainder rows.

Operand layouts (fp8 e4m3):
  e1dr [128, n_it*256]: per i-tile block of 256 cols: [0:128] =
    (-2*e1).T anchor block, [128:256] = ones on partitions 0-3.
  rhsA/rhsB [128, 8192]: per (g, s) chunk of 1024 cols at
    g*2048 + s*1024: [0:512] = pair columns (diff / base side),
    [512:1024] = bias plane (rows 0-3 = 4-term fp8 bias split).
"""

import os
import sys

for _p in ("/opt/trn_rl_repo",):
    if _p not in sys.path:
        sys.path.insert(0, _p)

import numpy as np
import ml_dtypes

EPS = 1e-6
MARGIN = 0.2
B = 8192
D = 128
NCORES = 8
NPAIR = 4096          # pair-columns per core (all cores see all pairs)
GW = 1024             # pair-cols per PSUM group = 2 banks
NG = NPAIR // GW      # 4 groups per i-tile
BIG = 1.0e30

_programs = {}
LAST_RESULTS = None   # BassKernelResults of the most recent run (for profiling)


def _build_program(n_it: int, pairb: int):
    """Bass program for one core.

    n_it: i-tiles (of 128 anchors) per core.
    pairb: pos/neg boundary in pair-column space.
    """
    import concourse.bacc as bacc
    import concourse.tile as tile
    from concourse import mybir

    f32 = mybir.dt.float32
    bf16 = mybir.dt.bfloat16
    fp8 = mybir.dt.float8e4
    AOT = mybir.AluOpType
    AFT = mybir.ActivationFunctionType
    DR = mybir.MatmulPerfMode.DoubleRow

    SH = n_it * 128

    nc = bacc.Bacc(None)
    e1dr = nc.declare_dram_parameter("e1dr", [D, 2 * SH], fp8, isOutput=False)
    rhsAe = nc.declare_dram_parameter("rhsAe", [D, NPAIR], fp8, isOutput=False)
    rhsBe = nc.declare_dram_parameter("rhsBe", [D, NPAIR], fp8, isOutput=False)
    rhsAb = nc.declare_dram_parameter("rhsAb", [32, NPAIR], fp8, isOutput=False)
    rhsBb = nc.declare_dram_parameter("rhsBb", [32, NPAIR], fp8, isOutput=False)
    outp = nc.declare_dram_parameter("out", [128, 2 * n_it], f32, isOutput=True)
    NCH = 2 * NG  # 8 chunks of 512 pair-cols

    def group_segs(g):
        """Class segments (lo, hi, is_pos) of group g in pair-col coords."""
        glo, ghi = g * GW, (g + 1) * GW
        segs = []
        if glo < pairb:
            segs.append((glo, min(ghi, pairb), True))
        if ghi > pairb:
            segs.append((max(glo, pairb), ghi, False))
        return segs

    with tile.TileContext(nc) as tc:
        with (
            tc.tile_pool(name="const", bufs=1) as cpool,
            tc.tile_pool(name="ps", bufs=4, space="PSUM") as pspool,
            tc.tile_pool(name="red", bufs=2) as redpool,
        ):
            e1sb = cpool.tile([D, 2 * SH], fp8, tag="e1sb")
            outsb = cpool.tile([128, 2 * n_it], f32, tag="outsb")
            trf = cpool.tile([128, GW], bf16, tag="trf")
            rhsAsb = cpool.tile([D, 2 * NPAIR], fp8, tag="rhsAsb")
            rhsBsb = cpool.tile([D, 2 * NPAIR], fp8, tag="rhsBsb")

            # Zero the bias-plane garbage rows once (rows 4-127 of the
            # i=1 planes multiply against zero weights but must not
            # contain fp8 NaN patterns).  uint32 bitcast -> 4x fewer
            # elements; DVE is idle early.
            # Layout: cols [0:NPAIR] = emb plane (pair order), cols
            # [NPAIR:2*NPAIR] = bias plane (rows 0-31 DMA'd, 32-127
            # zeroed).  All DMAs are plain contiguous 2D -- the earlier
            # 512B-interleaved layout ran the HBM reads at ~1.3 B/ns.
            Au = rhsAsb[:].bitcast(mybir.dt.uint32)
            Bu = rhsBsb[:].bitcast(mybir.dt.uint32)
            nc.gpsimd.memset(Au[:, NPAIR // 4:NPAIR // 2], 0)
            nc.gpsimd.memset(Bu[:, NPAIR // 4:NPAIR // 2], 0)

            # gen-0-critical pieces first, all on the fast sync queue;
            # B side follows on gpsimd; Act queue stays free for relus.
            nc.sync.dma_start(rhsAsb[:, 0:1024], rhsAe[:, 0:1024])
            nc.sync.dma_start(rhsAsb[0:32, NPAIR:2 * NPAIR], rhsAb[:])
            nc.sync.dma_start(e1sb[:], e1dr[:])
            nc.gpsimd.dma_start(rhsBsb[:, 0:1024], rhsBe[:, 0:1024])
            nc.gpsimd.dma_start(rhsBsb[0:32, NPAIR:2 * NPAIR], rhsBb[:])
            nc.sync.dma_start(rhsAsb[:, 1024:NPAIR], rhsAe[:, 1024:NPAIR])
            nc.gpsimd.dma_start(rhsBsb[:, 1024:NPAIR], rhsBe[:, 1024:NPAIR])

            w3full = e1sb[:].rearrange("p (i m) -> p i m", i=2)
            rA3 = rhsAsb[:].rearrange("p (i n) -> p i n", i=2)
            rB3 = rhsBsb[:].rearrange("p (i n) -> p i n", i=2)

            def emit_A(it, g, ps):
                w3 = w3full[:, :, it * 128:(it + 1) * 128]
                for s in range(2):
                    j0 = g * GW + s * 512
                    nc.tensor.matmul(
                        ps[:, s * 512:(s + 1) * 512],
                        w3,
                        rA3[:, :, j0:j0 + 512],
                        start=True,
                        stop=True,
                        perf_mode=DR,
                    )
                # relu in place (PSUM -> PSUM, has_written untouched)
                nc.scalar.activation(ps[:], ps[:], AFT.Relu)

            def emit_B(it, g, ps, chain):
                w3 = w3full[:, :, it * 128:(it + 1) * 128]
                for s in range(2):
                    j0 = g * GW + s * 512
                    nc.tensor.matmul(
                        ps[:, s * 512:(s + 1) * 512],
                        w3,
                        rB3[:, :, j0:j0 + 512],
                        start=False,
                        stop=True,
                        perf_mode=DR,
                        skip_group_check=True,
                    )
                # drain: plain max-reduce into a per-(gen, class) slot --
                # no accumulator readback, no cross-gen dependency.
                for lo, hi, is_pos in group_segs(g):
                    ll, lh = lo - g * GW, hi - g * GW
                    si = (0 if is_pos else NG) + g
                    nc.vector.tensor_reduce(
                        chain[:, si:si + 1],
                        ps[:, ll:lh],
                        axis=mybir.AxisListType.X,
                        op=AOT.max,
                    )
                if g == NG - 1:
                    # merge slots: [128, (class, gen)] -> [128, class]
                    nc.vector.tensor_reduce(
                        outsb[:, 2 * it:2 * it + 2],
                        chain[:].rearrange("p (c q) -> p c q", c=2),
                        axis=mybir.AxisListType.X,
                        op=AOT.max,
                    )

            # Software pipeline, depth 2: A(gen n+1), A(gen n+2) are
            # emitted BEFORE B(gen n) so the PE FIFO never head-of-line
            # blocks on a relu and the DVE stays saturated.
            chains = {}
            pending = []
            for it in range(n_it):
                ch = redpool.tile([128, 2 * NG], f32, tag="chain",
                                  name=f"chain_{it}")
                nc.vector.memset(ch[:], -BIG)
                chains[it] = ch
                for g in range(NG):
                    ps = pspool.tile([128, GW], f32, tag="ps", name=f"ps_{it}_{g}")
                    emit_A(it, g, ps)
                    pending.append((it, g, ps))
                    if len(pending) > 2:
                        pit, pg, pps = pending.pop(0)
                        emit_B(pit, pg, pps, chains[pit])
            for pit, pg, pps in pending:
                emit_B(pit, pg, pps, chains[pit])
            nc.sync.dma_start(outp[:], outsb[:])
    nc.compile()
    return nc


def _fp8_split4(x):
    """4-term fp8 e4m3 split of x (f64): returns [4, n] planes whose sum
    approximates x to ~1e-3 absolute (saturates at +-448*4)."""
    terms = []
    rem = x.astype(np.float64).copy()
    for _ in range(4):
        t = rem.astype(np.float32).astype(ml_dtypes.float8_e4m3)
        terms.append(t)
        rem = rem - t.astype(np.float64)
    return np.stack(terms)


def _host_prep(emb1, emb2, target):
    """Sort columns pos-first, build pairs, pack device operands."""
    tpos = target == 1
    k = int(tpos.sum())
    perm = np.concatenate([np.nonzero(tpos)[0], np.nonzero(~tpos)[0]])
    e2s = emb2[perm].astype(np.float64)          # [B, D] sorted pos-first
    b = (e2s * e2s).sum(1) - (2.0 * EPS) * e2s.sum(1)

    nneg = B - k
    peel_pos = k % 2
    peel_neg = nneg % 2
    k2, n2 = k - peel_pos, nneg - peel_neg
    peeled = []
    if peel_pos:
        peeled.append((e2s[k - 1], b[k - 1], True))
    if peel_neg:
        peeled.append((e2s[B - 1], b[B - 1], False))

    npairs_pos = k2 // 2
    npairs_neg = n2 // 2
    ndum = NPAIR - npairs_pos - npairs_neg
    assert ndum >= 0

    dA = np.zeros((NPAIR, D))                    # rhsA pair columns (diff)
    dB = np.zeros((NPAIR, D))                    # rhsB pair columns (base)
    bA = np.zeros(NPAIR)
    bB = np.full(NPAIR, -4.0 * 448.0)            # dummy: 4x fp8-saturated
    # pos: A = d_u - d_v ; B = d_v
    u = e2s[0:k2:2]
    v = e2s[1:k2:2]
    dA[:npairs_pos] = u - v
    dB[:npairs_pos] = v
    bA[:npairs_pos] = b[0:k2:2] - b[1:k2:2]
    bB[:npairs_pos] = b[1:k2:2]
    # neg (sign-flipped): A = d_v - d_u ; B = -d_v
    nu = e2s[k:k + n2:2]
    nv = e2s[k + 1:k + n2:2]
    sl = slice(npairs_pos, npairs_pos + npairs_neg)
    dA[sl] = nv - nu
    dB[sl] = -nv
    bA[sl] = b[k + 1:k + n2:2] - b[k:k + n2:2]
    bB[sl] = -b[k + 1:k + n2:2]

    e1p = emb1[tpos]                             # [k, D] pos anchors
    e1d = e1p.astype(np.float64)
    a = (e1d * e1d).sum(1) + (2.0 * EPS) * e1d.sum(1) + D * EPS * EPS

    n_it = min(k // 1024, 8)
    ndev = n_it * 1024
    SH = n_it * 128

    # e1dr: per-core built later (anchor blocks per core); build full here
    e1m2t = (-2.0 * e1p[:ndev]).T.astype(np.float32)   # [D, ndev]

    def pack_rhs(cols, bias):
        colsT = np.ascontiguousarray(cols.T).astype(np.float32).astype(
            ml_dtypes.float8_e4m3)                  # [D, NPAIR]
        bias32 = np.zeros((32, NPAIR), dtype=ml_dtypes.float8_e4m3)
        bias32[0:4] = _fp8_split4(bias)
        return colsT, bias32

    rhsAe8, rhsAb8 = pack_rhs(dA, bA)
    rhsBe8, rhsBb8 = pack_rhs(dB, bB)
    pairb = npairs_pos
    return (k, n_it, a, e1p, pairb, e1m2t,
            rhsAe8, rhsAb8, rhsBe8, rhsBb8, peeled)


def _pack_e1dr(e1m2t_core, n_it):
    """[D, SH] f32 -> [D, 2*SH] fp8: [0:SH] = emb, [SH:2SH] = ones rows."""
    SH = n_it * 128
    out = np.zeros((D, 2 * SH), dtype=ml_dtypes.float8_e4m3)
    out[:, 0:SH] = e1m2t_core.astype(ml_dtypes.float8_e4m3)
    out[0:4, SH:2 * SH] = 1.0
    return out


def _host_remainder(e1rem, emb2, target):
    """Exact f64 pos_max/neg_min contribution of the remainder anchors."""
    e1d = e1rem.astype(np.float64)
    e2d = emb2.astype(np.float64)
    sq = (
        (e1d * e1d).sum(1)[:, None]
        + (e2d * e2d).sum(1)[None, :]
        - 2.0 * (e1d @ e2d.T)
        + 2.0 * EPS * (e1d.sum(1)[:, None] - e2d.sum(1)[None, :])
        + D * EPS * EPS
    )
    dist = np.sqrt(np.clip(sq, 0.0, None))
    pos = target == 1
    pos_max = np.where(pos[None, :], dist, -np.inf).max(1)
    neg_min = np.where(~pos[None, :], dist, np.inf).min(1)
    return np.clip(pos_max - neg_min + MARGIN, 0.0, None).sum()


def _numpy_fallback(emb1, emb2, target):
    e1 = emb1.astype(np.float64)
    e2 = emb2.astype(np.float64)
    sq = (
        (e1 * e1).sum(1)[:, None]
        + (e2 * e2).sum(1)[None, :]
        - 2.0 * (e1 @ e2.T)
        + 2.0 * EPS * (e1.sum(1)[:, None] - e2.sum(1)[None, :])
        + D * EPS * EPS
    )
    dist = np.sqrt(np.clip(sq, 0.0, None))
    pos = target == 1
    neg = target == 0
    pos_max = np.where(pos[None, :], dist, -np.inf).max(1)
    neg_min = np.where(neg[None, :], dist, np.inf).min(1)
    per = np.maximum(pos_max - neg_min + MARGIN, 0.0)
    w = pos.astype(np.float64)
    return np.float32((per * w).sum() / w.sum())


def kernel(emb1, emb2, target):
    global LAST_RESULTS
    emb1 = np.asarray(emb1, dtype=np.float32)
    emb2 = np.asarray(emb2, dtype=np.float32)
    target = np.asarray(target)
    assert emb1.shape == (B, D) and emb2.shape == (B, D)

    k = int((target == 1).sum())
    if k < 1024 or k == B:
        return _numpy_fallback(emb1, emb2, target)

    (k, n_it, a, e1p, pairb, e1m2t, rhsAe8, rhsAb8, rhsBe8, rhsBb8,
     peeled) = _host_prep(emb1, emb2, target)
    ndev = n_it * 1024
    SH = n_it * 128

    nc = _programs.get((n_it, pairb))
    if nc is None:
        nc = _build_program(n_it, pairb)
        _programs[(n_it, pairb)] = nc

    from concourse.bass_utils import run_bass_kernel_spmd

    in_maps = [
        {
            "e1dr": _pack_e1dr(e1m2t[:, c * SH:(c + 1) * SH], n_it),
            "rhsAe": rhsAe8,
            "rhsAb": rhsAb8,
            "rhsBe": rhsBe8,
            "rhsBb": rhsBb8,
        }
        for c in range(NCORES)
    ]
    res = run_bass_kernel_spmd(nc, in_maps, core_ids=list(range(NCORES)))
    LAST_RESULTS = res

    Mp = np.concatenate(
        [np.asarray(res.results[c]["out"])[:, 0::2].T.reshape(-1)
         for c in range(NCORES)]
    ).astype(np.float64)
    Mn = np.concatenate(
        [np.asarray(res.results[c]["out"])[:, 1::2].T.reshape(-1)
         for c in range(NCORES)]
    ).astype(np.float64)

    # merge peeled columns exactly (host f64)
    e1d = e1p[:ndev].astype(np.float64)
    for col, bias, is_pos in peeled:
        vj = bias - 2.0 * (e1d @ col)
        if is_pos:
            Mp = np.maximum(Mp, vj)
        else:
            Mn = np.maximum(Mn, -vj)

    adev = a[:ndev]
    pos2 = np.clip(adev + Mp, 0.0, None)
    neg2 = np.clip(adev - Mn, 0.0, None)   # min v = -max(-v)
    per = np.clip(np.sqrt(pos2) - np.sqrt(neg2) + MARGIN, 0.0, None)
    total = per.sum()
    if ndev < k:
        total += _host_remainder(e1p[ndev:], emb2, target)
    return np.float32(total / k)
